# revision 55
# baseline (speedup 1.0000x reference)
"""Trainium2 Bass kernel for nn_Attention_12369505813001.

Computes, per batch b:
    qw    = query @ W_in.T                      [T, H]
    score = qw @ enc.T                          [T, S]
    p     = softmax(mask(score), axis=S)
    c     = p @ enc                             [T, H]
    out   = tanh(concat(query, c) @ W_out.T + b_out)

Shapes: B=32, T=512, S=1024, H=1024, fp32. Data-parallel over B across
8 NeuronCores (4 batches/core); no collectives.

Layout strategy (per core): keep the feature dim on partitions and T on
the free axis throughout ("transposed" layouts), so the PE contraction
dim always lands on partitions and no on-device transposes are needed.

Precision: the softmax path needs ~14+ effective mantissa bits (the
softmax exponentiates score errors; score std is sqrt(H)=32), which no
single-pass PE dtype provides. Steps 1-2 therefore run a two-pass
scheme per 128x128 tile at ~16 effective bits:
    hi@hi   in fp16 (11-bit mantissa, 1 row/cycle), moving operand
            pre-scaled by 2^12
    cross   hi@lo + lo@hi packed into ONE fp8-e4m3 DoubleRow matmul
            (two virtual k-subtiles, 0.5 rows/cycle); per-slot scale
            factors are chosen so both products also come out at 2^12
Both accumulate into the SAME fp32 PSUM bank; one Copy-activation with
scale=2^-12 recovers the fp32 result. The correction terms are 2^-12
of the main term, so fp8's 4-bit relative accuracy on them lands at
~2^-16. The context (step 4) runs in plain bf16 with e stored bf16
(denominator computed from the same quantized e preserves the softmax
simplex), and step 5 runs in fp16. CPU-simulated end-to-end rel err of
this exact scheme: 8.4e-3 (1.1e-2 if the PE flushes fp16 subnormals);
gate is 2e-2.

Scheduling: per-engine instruction streams are ordered at compile
time, so the batch loop is software-pipelined by hand into
A(load+step1+step2) / B(softmax+den) / C(context+step5) phases emitted
as A0 A1 B0 C0 B1 A2 C1 B2 A3 C2 B3 C3 — every B's vector/scalar
softmax work has another batch's matmuls in flight on the PE.
The denominator uses an all-ones [128x128] stationary matmul so den
lands broadcast on all 128 partitions (full-lane DVE reciprocal, no
partition ops). DMAs are spread across both HWDGE queues with 1-2KB
per-partition lines (host pre-tiles eT accordingly).
"""

from contextlib import ExitStack

import numpy as np
import ml_dtypes

import concourse.bass as bass
import concourse.bass_isa as bass_isa
import concourse.mybir as mybir
import concourse.tile as tile
from concourse import bacc
from concourse.bass_utils import run_bass_kernel_spmd

B, T, S, H = 32, 512, 1024, 1024
NCORES = 8
BPC = B // NCORES          # batches per core
HT = H // 128              # h/o chunk count
ST = S // 128              # s chunk count
P = 128

f32 = mybir.dt.float32
bf16 = mybir.dt.bfloat16
f16 = mybir.dt.float16
fp8 = mybir.dt.float8e4
AX = mybir.AxisListType.X
AF = mybir.ActivationFunctionType
ALU = mybir.AluOpType
DR = mybir.MatmulPerfMode.DoubleRow

MASKVAL = -1.0e38
SC = 2.0 ** 12             # shared product scale of the split matmuls

_nc_cache_by_cps = {}

TRACE = False          # set by test.py to capture an NTFF/perfetto profile
LAST_RESULTS = None    # test.py reads exec_time_ns / trace path from here


def _build_nc(cps):
    """cps[j] = number of live 128-row s-chunks for batch slot j (fully
    masked chunks contribute exactly 0 to softmax/context and are skipped;
    the host sorts batches by length so every core's slot-j batch fits)."""
    nc = bacc.Bacc("TRN2", target_bir_lowering=False, debug=False)

    q16s_d = nc.dram_tensor("q16s", [BPC, H, T], f16, kind="ExternalInput")
    q8_d = nc.dram_tensor("q8", [BPC, H, 2, T], fp8, kind="ExternalInput")
    q16_d = nc.dram_tensor("q16", [BPC, H, T], f16, kind="ExternalInput")
    # eT pre-tiled and byte-packed on host: [b, k, p, m2, 2, 512] uint8 where
    # [..., 0, :] holds 256 fp16 hi values and [..., 1, :] the 2x256 fp8
    # correction slots — one DMA per (k, m2) with 1KB per-partition lines
    eTp = nc.dram_tensor("eTp", [BPC, HT, P, ST // 2, 2, 512], mybir.dt.uint8,
                         kind="ExternalInput")
    encb = nc.dram_tensor("encb", [BPC, S, H], bf16, kind="ExternalInput")
    maskc = nc.dram_tensor("maskc", [BPC, P, ST], f32, kind="ExternalInput")
    W1h_d = nc.dram_tensor("W1h", [H, H], f16, kind="ExternalInput")
    W1c_d = nc.dram_tensor("W1c", [H, 2, H], fp8, kind="ExternalInput")
    W16 = nc.dram_tensor("W16", [H, 2, H], f16, kind="ExternalInput")
    bo = nc.dram_tensor("bo", [P, HT], f32, kind="ExternalInput")
    onesv = nc.dram_tensor("onesv", [P, P], bf16, kind="ExternalInput")
    outT = nc.dram_tensor("outT", [BPC, H, T], f32, kind="ExternalOutput")

    with tile.TileContext(nc) as tc, ExitStack() as ctx:
        wp = ctx.enter_context(tc.tile_pool(name="wp", bufs=1))
        pq = ctx.enter_context(tc.tile_pool(name="pq", bufs=2))
        pq16 = ctx.enter_context(tc.tile_pool(name="pq16", bufs=1))
        pb = ctx.enter_context(tc.tile_pool(name="pb", bufs=2))
        sp = ctx.enter_context(tc.tile_pool(name="sp", bufs=2))
        sp1 = ctx.enter_context(tc.tile_pool(name="sp1", bufs=1))
        etp = ctx.enter_context(tc.tile_pool(name="etp", bufs=10))
        enc_p = ctx.enter_context(tc.tile_pool(name="enc_p", bufs=1))
        otp = ctx.enter_context(tc.tile_pool(name="otp", bufs=2))
        psQ = ctx.enter_context(tc.tile_pool(name="psQ", bufs=2, space="PSUM"))
        psO = ctx.enter_context(tc.tile_pool(name="psO", bufs=2, space="PSUM"))
        psC = ctx.enter_context(tc.tile_pool(name="psC", bufs=3, space="PSUM"))
        psD = ctx.enter_context(tc.tile_pool(name="psD", bufs=1, space="PSUM"))

        # --- persistent weights, interleaved per-k with batch 0's q load so
        # the first step-1 matmul starts after ~1/8 of the weight traffic
        # (w16 is emitted later: first use is C0) ---
        w1h = wp.tile([P, HT, H], f16)
        w1c = wp.tile([P, HT, 2, H], fp8)
        q16s0 = pq.tile([P, HT, T], f16, tag="q16s")
        q8s0 = pq.tile([P, HT, 2, T], fp8, tag="q8")
        # HAM warmup: the PE clock sits at 1.2 GHz until it has been busy
        # for a full ~3.4us activity window.  Real matmuls can't start until
        # the first weight/q chunks land (~12us of DMA), and that trickle
        # keeps the clock throttled for the first ~25us.  Burn the wait on
        # dummy matmuls over uninitialized scratch (no input deps, result
        # never read) so the array is at 2.4 GHz when real work starts.
        warm_w = wp.tile([P, P], bf16)
        warm_m = wp.tile([P, T], bf16)
        nc.scalar.memzero(warm_w)
        nc.scalar.memzero(warm_m)
        for i in range(18):
            warm_ps = psQ.tile([P, T], f32, tag="qs", name=f"warm_{i}")
            nc.tensor.matmul(warm_ps, warm_w, warm_m, start=True, stop=True)

        # fp16 operands first: the grouped-per-chunk matmul order runs all
        # fp16 matmuls before the fp8 corrections, so step 1 can start as
        # soon as w1h/q16s land.  All DMAs issue from the Sync engine only —
        # a DMA trigger blocked on a busy ring would stall any compute op
        # behind it in that engine's FIFO, and ACT/DVE drains are on the
        # critical path.
        # startup-only: issue from both engines in parallel (ACT has no
        # compute queued yet, so its FIFO can't block anything critical)
        for k in range(HT):
            ksl = slice(128 * k, 128 * (k + 1))
            nc.sync.dma_start(out=w1h[:, k, :], in_=W1h_d[ksl, :])
            nc.scalar.dma_start(out=q16s0[:, k, :], in_=q16s_d[0, ksl, :])
        for k in range(HT):
            ksl = slice(128 * k, 128 * (k + 1))
            nc.sync.dma_start(out=w1c[:, k, :, :], in_=W1c_d[ksl, :, :])
            nc.scalar.dma_start(out=q8s0[:, k, :, :], in_=q8_d[0, ksl, :, :])
        bo_sb = wp.tile([P, HT], f32)
        nc.sync.dma_start(out=bo_sb, in_=bo[:, :])
        mask_sb = wp.tile([P, BPC, ST], f32)
        nc.sync.dma_start(out=mask_sb, in_=maskc[:, :, :].rearrange("b p m -> p b m"))
        ones_sb = wp.tile([P, P], bf16)
        nc.sync.dma_start(out=ones_sb, in_=onesv[:, :])
        w16 = wp.tile([P, 2, HT, H], f16)

        st = {}

        def load_q16(b):
            # step-5 moving q for batch b; emitted well before C(b) so the
            # transfer isn't stuck behind a later batch's demand-paced et
            # stream
            q16 = pq16.tile([P, HT, T], f16, tag="q16")
            nc.sync.dma_start(
                out=q16,
                in_=q16_d[b, :, :].rearrange("(k p) t -> p k t", p=P))
            st[("q16", b)] = q16

        def phase_A(b):
            # --- load q side; step 1; step 2 + max tree ---
            if b == 0:
                q16s, q8s = q16s0, q8s0
            else:
                q16s = pq.tile([P, HT, T], f16, tag="q16s")
                q8s = pq.tile([P, HT, 2, T], fp8, tag="q8")
                nc.sync.dma_start(
                    out=q16s,
                    in_=q16s_d[b, :, :].rearrange("(k p) t -> p k t", p=P))
                nc.sync.dma_start(
                    out=q8s,
                    in_=q8_d[b, :, :, :].rearrange("(k p) c t -> p k c t", p=P))
            if b > 0:
                load_q16(b - 1)

            # step 1: PSUM accumulates 2^12 * qw
            qwh = pb.tile([P, HT, T], f16, tag="big2a")      # fp16(2^12 qw)
            qw8c = pb.tile([P, HT, 2, T], fp8, tag="big2b")  # [4096*qwlo, qw]
            for m in range(HT):
                qw_ps = psQ.tile([P, T], f32, tag="qs", name=f"qw_{b}_{m}")
                msl = slice(128 * m, 128 * (m + 1))
                for k in range(HT):
                    nc.tensor.matmul(qw_ps, w1h[:, k, msl], q16s[:, k, :],
                                     start=(k == 0), stop=False)
                for k in range(HT):
                    nc.tensor.matmul(qw_ps, w1c[:, k, :, msl], q8s[:, k, :, :],
                                     perf_mode=DR,
                                     start=False, stop=(k == HT - 1))
                nc.scalar.copy(qwh[:, m, :], qw_ps)
                nc.scalar.activation(qw8c[:, m, 1, :], qw_ps, AF.Copy,
                                     scale=1.0 / SC)
                nc.vector.tensor_sub(qw8c[:, m, 0, :], qw_ps, qwh[:, m, :])

            # step 2: PSUM accumulates 2^12 * score
            L = cps[b]
            score = pb.tile([P, ST, T], f32, tag="big1")
            smax = sp.tile([P, T], f32, tag="smax")
            for m2 in range((L + 1) // 2):
                ets16, ets8 = [], []
                for k in range(HT):
                    et = etp.tile([P, 2, 512], mybir.dt.uint8, tag="et")
                    nc.sync.dma_start(out=et, in_=eTp[b, k, :, m2, :, :])
                    ets16.append(et[:, 0, :].bitcast(f16))
                    ets8.append(et[:, 1, :].bitcast(fp8)
                                .rearrange("p (c x) -> p c x", c=2))
                for j in range(2):
                    m = 2 * m2 + j
                    if m >= L:
                        continue
                    jsl = slice(128 * j, 128 * (j + 1))
                    sc_ps = psQ.tile([P, T], f32, tag="qs", name=f"sc_{b}_{m}")
                    for k in range(HT):
                        nc.tensor.matmul(sc_ps, ets16[k][:, jsl], qwh[:, k, :],
                                         start=(k == 0), stop=False)
                    for k in range(HT):
                        nc.tensor.matmul(sc_ps, ets8[k][:, :, jsl],
                                         qw8c[:, k, :, :], perf_mode=DR,
                                         start=False, stop=(k == HT - 1))
                    nc.scalar.activation(score[:, m, :], sc_ps, AF.Copy,
                                         scale=1.0 / SC)
                    if m == 0:
                        nc.vector.tensor_copy(smax, score[:, m, :])
                    else:
                        nc.vector.tensor_max(smax, smax, score[:, m, :])
            st[b] = (score, smax)

        def phase_B(b):
            # --- softmax: global max, exp (-> bf16 e), denominator ---
            score, smax = st[b]
            # max over masked-but-real rows is included; the uniform upward
            # shift cancels in the softmax ratio.
            smax_all = sp1.tile([P, T], f32, tag="smax_all")
            nc.gpsimd.partition_all_reduce(smax_all, smax, channels=P,
                                           reduce_op=bass_isa.ReduceOp.max)
            L = cps[b]
            e = pb.tile([P, ST, T], bf16, tag="big2a")
            for m in range(L):
                nc.vector.tensor_sub(score[:, m, :], score[:, m, :], smax_all)
                nc.scalar.activation(e[:, m, :], score[:, m, :], AF.Exp,
                                     bias=mask_sb[:, b, m:m + 1])
            # all-ones stationary -> den replicated on all 128 partitions
            den_ps = psD.tile([P, T], f32, tag="den", name=f"den_{b}")
            for m in range(L):
                nc.tensor.matmul(den_ps, ones_sb, e[:, m, :],
                                 start=(m == 0), stop=(m == L - 1))
            rdenb = sp1.tile([P, T], f32, tag="rdenb")
            nc.vector.reciprocal(rdenb, den_ps)
            st[b] = (e, rdenb)

        def phase_C(b):
            # --- context (bf16) + output projection (fp16) ---
            L = cps[b]
            e, rdenb = st[b]
            del st[b]
            q16 = st.pop(("q16", b))
            enc_sb = enc_p.tile([P, ST, H], bf16, tag="enc")
            for k in range(L):
                nc.sync.dma_start(out=enc_sb[:, k, :],
                                  in_=encb[b, 128 * k:128 * (k + 1), :])

            cn = pb.tile([P, HT, T], f16, tag="big1")
            for m in range(HT):
                c_ps = psC.tile([P, T], f32, tag="c", name=f"c_{b}_{m}")
                for k in range(L):
                    nc.tensor.matmul(c_ps, enc_sb[:, k, 128 * m:128 * (m + 1)],
                                     e[:, k, :],
                                     start=(k == 0), stop=(k == L - 1))
                nc.vector.tensor_mul(cn[:, m, :], c_ps, rdenb)

            for m in range(HT):
                o_ps = psO.tile([P, T], f32, tag="o", name=f"o_{b}_{m}")
                msl = slice(128 * m, 128 * (m + 1))
                for k in range(HT):
                    nc.tensor.matmul(o_ps, w16[:, 0, k, msl], q16[:, k, :],
                                     start=(k == 0), stop=False)
                for k in range(HT):
                    nc.tensor.matmul(o_ps, w16[:, 1, k, msl], cn[:, k, :],
                                     start=False, stop=(k == HT - 1))
                ot = otp.tile([P, T], f32, tag="ot")
                nc.scalar.activation(ot, o_ps, AF.Tanh, bias=bo_sb[:, m:m + 1])
                nc.sync.dma_start(out=outT[b, 128 * m:128 * (m + 1), :],
                                  in_=ot)

        # software pipeline: A0 A1 B0 C0 B1 A2 C1 B2 A3 C2 B3 C3
        phase_A(0)
        nc.sync.dma_start(
            out=w16, in_=W16[:, :, :].rearrange("(k p) c o -> p c k o", p=P))
        phase_A(1)
        phase_B(0)
        phase_C(0)
        phase_B(1)
        phase_A(2)
        phase_C(1)
        phase_B(2)
        phase_A(3)
        phase_C(2)
        load_q16(3)
        phase_B(3)
        phase_C(3)

    nc.compile()
    return nc


def _f16_split(x):
    hi = x.astype(np.float16).astype(np.float32)
    return hi, x - hi


def _f8(x):
    return x.astype(ml_dtypes.float8_e4m3)


def kernel(query, encoder_outputs, src_lengths, W_in, W_out, b_out):
    query = np.asarray(query, dtype=np.float32)
    encoder_outputs = np.ascontiguousarray(np.asarray(encoder_outputs, np.float32))
    src_lengths = np.asarray(src_lengths)
    W_in = np.asarray(W_in, dtype=np.float32)
    W_out = np.asarray(W_out, dtype=np.float32)
    b_out = np.asarray(b_out, dtype=np.float32)

    # --- shared (weight) inputs ---
    W_inT = np.ascontiguousarray(W_in.T)                    # [h, o]
    _, Wlo = _f16_split(W_inT)
    W1h = W_inT.astype(np.float16)
    W1c = np.ascontiguousarray(
        np.stack([_f8(8.0 * W_inT), _f8(16384.0 * Wlo)], axis=1))  # [h, 2, o]
    W16 = np.ascontiguousarray(
        np.stack([W_out[:, :H].T, W_out[:, H:].T], axis=1)  # [h, 2, o]
    ).astype(np.float16)
    bo = np.ascontiguousarray(b_out.reshape(HT, P).T)       # [p, m]
    onesv = np.ones((P, P), dtype=ml_dtypes.bfloat16)

    # --- batch -> (core, slot) assignment: sort by length (desc) so slot j
    # holds ranks [8j, 8j+8) and the compiled per-slot chunk count is the
    # max over one octile, minimizing skipped-chunk waste ---
    lens_all = np.asarray(src_lengths, dtype=np.int64)
    order = np.argsort(-lens_all, kind="stable")            # [32] batch ids
    asg = order.reshape(BPC, NCORES)                        # [slot, core]
    cps = tuple(int(np.ceil(lens_all[asg[j]].max() / P)) for j in range(BPC))

    # --- per-core shards ---
    in_maps = []
    for c in range(NCORES):
        bidx = asg[:, c]                                    # batch ids, by slot
        q = query[bidx]                                     # [BPC, T, H]
        encs = np.ascontiguousarray(encoder_outputs[bidx])  # [BPC, S, H]
        lens = lens_all[bidx]

        qTa = np.ascontiguousarray(q.transpose(0, 2, 1))    # [BPC, H, T]
        _, qlo = _f16_split(qTa)
        q16sa = (4096.0 * qTa).astype(np.float16)           # fp16(2^12 q)
        q8a = np.ascontiguousarray(
            np.stack([_f8(512.0 * qlo), _f8(0.25 * qTa)], axis=2))  # [b,h,2,t]
        q16a = qTa.astype(np.float16)                       # [BPC, H, T]
        encTa = np.ascontiguousarray(encs.transpose(0, 2, 1))  # [BPC, H, S]
        eh, elo = _f16_split(encTa)
        # pre-tile + byte-pack fp16 hi and fp8 correction slots for one DMA
        # per (k, m2) with contiguous 1KB per-partition lines
        eT16a = eh.astype(np.float16).reshape(BPC, HT, P, ST // 2, 256)
        e8hi = _f8(encTa).reshape(BPC, HT, P, ST // 2, 256)
        e8lo = _f8(4096.0 * elo).reshape(BPC, HT, P, ST // 2, 256)
        eT8a = np.stack([e8hi, e8lo], axis=4)       # [b,k,p,m2,2,256]
        eTpa = np.empty((BPC, HT, P, ST // 2, 2, 512), dtype=np.uint8)
        eTpa[..., 0, :] = eT16a.view(np.uint8).reshape(BPC, HT, P, ST // 2, 512)
        eTpa[..., 1, :] = eT8a.view(np.uint8).reshape(BPC, HT, P, ST // 2, 512)
        encba = encs.astype(ml_dtypes.bfloat16)             # [BPC, S, H]

        maskca = np.zeros((BPC, P, ST), dtype=np.float32)
        pos = (np.arange(ST)[None, :] * P + np.arange(P)[:, None])  # [P, ST]
        for j in range(BPC):
            maskca[j][pos >= lens[j]] = MASKVAL

        in_maps.append({
            "q16s": q16sa, "q8": q8a, "q16": q16a,
            "eTp": eTpa, "encb": encba,
            "maskc": maskca, "W1h": W1h, "W1c": W1c, "W16": W16,
            "bo": bo, "onesv": onesv,
        })

    if cps not in _nc_cache_by_cps:
        _nc_cache_by_cps[cps] = _build_nc(cps)
    nc = _nc_cache_by_cps[cps]

    res = run_bass_kernel_spmd(nc, in_maps, core_ids=list(range(NCORES)),
                               trace=TRACE)
    global LAST_RESULTS
    LAST_RESULTS = res

    out = np.empty((B, T, H), dtype=np.float32)
    for c in range(NCORES):
        o = res.results[c]["outT"]                          # [BPC, H, T]
        out[asg[:, c]] = o.transpose(0, 2, 1)
    return out


# revision 58
# speedup vs baseline: 1.0068x; 1.0068x over previous
"""Trainium2 Bass kernel for nn_Attention_12369505813001.

Computes, per batch b:
    qw    = query @ W_in.T                      [T, H]
    score = qw @ enc.T                          [T, S]
    p     = softmax(mask(score), axis=S)
    c     = p @ enc                             [T, H]
    out   = tanh(concat(query, c) @ W_out.T + b_out)

Shapes: B=32, T=512, S=1024, H=1024, fp32. Data-parallel over B across
8 NeuronCores (4 batches/core); no collectives.

Layout strategy (per core): keep the feature dim on partitions and T on
the free axis throughout ("transposed" layouts), so the PE contraction
dim always lands on partitions and no on-device transposes are needed.

Precision: the softmax path needs ~14+ effective mantissa bits (the
softmax exponentiates score errors; score std is sqrt(H)=32), which no
single-pass PE dtype provides. Steps 1-2 therefore run a two-pass
scheme per 128x128 tile at ~16 effective bits:
    hi@hi   in fp16 (11-bit mantissa, 1 row/cycle), moving operand
            pre-scaled by 2^12
    cross   hi@lo + lo@hi packed into ONE fp8-e4m3 DoubleRow matmul
            (two virtual k-subtiles, 0.5 rows/cycle); per-slot scale
            factors are chosen so both products also come out at 2^12
Both accumulate into the SAME fp32 PSUM bank; one Copy-activation with
scale=2^-12 recovers the fp32 result. The correction terms are 2^-12
of the main term, so fp8's 4-bit relative accuracy on them lands at
~2^-16. The context (step 4) runs in plain bf16 with e stored bf16
(denominator computed from the same quantized e preserves the softmax
simplex), and step 5 runs in fp16. CPU-simulated end-to-end rel err of
this exact scheme: 8.4e-3 (1.1e-2 if the PE flushes fp16 subnormals);
gate is 2e-2.

Scheduling: per-engine instruction streams are ordered at compile
time, so the batch loop is software-pipelined by hand into
A(load+step1+step2) / B(softmax+den) / C(context+step5) phases emitted
as A0 A1 B0 C0 B1 A2 C1 B2 A3 C2 B3 C3 — every B's vector/scalar
softmax work has another batch's matmuls in flight on the PE.
The denominator uses an all-ones [128x128] stationary matmul so den
lands broadcast on all 128 partitions (full-lane DVE reciprocal, no
partition ops). DMAs are spread across both HWDGE queues with 1-2KB
per-partition lines (host pre-tiles eT accordingly).
"""

from contextlib import ExitStack

import numpy as np
import ml_dtypes

import concourse.bass as bass
import concourse.bass_isa as bass_isa
import concourse.mybir as mybir
import concourse.tile as tile
from concourse import bacc
from concourse.bass_utils import run_bass_kernel_spmd

B, T, S, H = 32, 512, 1024, 1024
NCORES = 8
BPC = B // NCORES          # batches per core
HT = H // 128              # h/o chunk count
ST = S // 128              # s chunk count
P = 128

f32 = mybir.dt.float32
bf16 = mybir.dt.bfloat16
f16 = mybir.dt.float16
fp8 = mybir.dt.float8e4
AX = mybir.AxisListType.X
AF = mybir.ActivationFunctionType
ALU = mybir.AluOpType
DR = mybir.MatmulPerfMode.DoubleRow

MASKVAL = -1.0e38
SC = 2.0 ** 12             # shared product scale of the split matmuls

_nc_cache_by_cps = {}

TRACE = False          # set by test.py to capture an NTFF/perfetto profile
LAST_RESULTS = None    # test.py reads exec_time_ns / trace path from here


def _build_nc(cps):
    """cps[j] = number of live 128-row s-chunks for batch slot j (fully
    masked chunks contribute exactly 0 to softmax/context and are skipped;
    the host sorts batches by length so every core's slot-j batch fits)."""
    nc = bacc.Bacc("TRN2", target_bir_lowering=False, debug=False)

    q16s_d = nc.dram_tensor("q16s", [BPC, H, T], f16, kind="ExternalInput")
    q8_d = nc.dram_tensor("q8", [BPC, H, 2, T], fp8, kind="ExternalInput")
    q16_d = nc.dram_tensor("q16", [BPC, H, T], f16, kind="ExternalInput")
    # eT pre-tiled and byte-packed on host: [b, k, p, m2, 2, 512] uint8 where
    # [..., 0, :] holds 256 fp16 hi values and [..., 1, :] the 2x256 fp8
    # correction slots — one DMA per (k, m2) with 1KB per-partition lines
    eTp = nc.dram_tensor("eTp", [BPC, HT, P, ST // 2, 2, 512], mybir.dt.uint8,
                         kind="ExternalInput")
    encb = nc.dram_tensor("encb", [BPC, S, H], bf16, kind="ExternalInput")
    maskc = nc.dram_tensor("maskc", [BPC, P, ST], f32, kind="ExternalInput")
    W1h_d = nc.dram_tensor("W1h", [H, H], f16, kind="ExternalInput")
    W1c_d = nc.dram_tensor("W1c", [H, 2, H], fp8, kind="ExternalInput")
    W16 = nc.dram_tensor("W16", [H, 2, H], f16, kind="ExternalInput")
    bo = nc.dram_tensor("bo", [P, HT], f32, kind="ExternalInput")
    onesv = nc.dram_tensor("onesv", [P, P], bf16, kind="ExternalInput")
    outT = nc.dram_tensor("outT", [BPC, H, T], f32, kind="ExternalOutput")

    with tile.TileContext(nc) as tc, ExitStack() as ctx:
        wp = ctx.enter_context(tc.tile_pool(name="wp", bufs=1))
        pq = ctx.enter_context(tc.tile_pool(name="pq", bufs=2))
        pq16 = ctx.enter_context(tc.tile_pool(name="pq16", bufs=1))
        pb = ctx.enter_context(tc.tile_pool(name="pb", bufs=2))
        sp = ctx.enter_context(tc.tile_pool(name="sp", bufs=2))
        sp1 = ctx.enter_context(tc.tile_pool(name="sp1", bufs=1))
        etp = ctx.enter_context(tc.tile_pool(name="etp", bufs=10))
        enc_p = ctx.enter_context(tc.tile_pool(name="enc_p", bufs=1))
        otp = ctx.enter_context(tc.tile_pool(name="otp", bufs=2))
        psQ = ctx.enter_context(tc.tile_pool(name="psQ", bufs=2, space="PSUM"))
        psO = ctx.enter_context(tc.tile_pool(name="psO", bufs=2, space="PSUM"))
        psC = ctx.enter_context(tc.tile_pool(name="psC", bufs=3, space="PSUM"))
        psD = ctx.enter_context(tc.tile_pool(name="psD", bufs=1, space="PSUM"))

        # --- persistent weights, interleaved per-k with batch 0's q load so
        # the first step-1 matmul starts after ~1/8 of the weight traffic
        # (w16 is emitted later: first use is C0) ---
        w1h = wp.tile([P, HT, H], f16)
        w1c = wp.tile([P, HT, 2, H], fp8)
        q16s0 = pq.tile([P, HT, T], f16, tag="q16s")
        q8s0 = pq.tile([P, HT, 2, T], fp8, tag="q8")
        # HAM warmup: the PE clock sits at 1.2 GHz until it has been busy
        # for a full ~3.4us activity window.  Real matmuls can't start until
        # the first weight/q chunks land (~12us of DMA), and that trickle
        # keeps the clock throttled for the first ~25us.  Burn the wait on
        # dummy matmuls over uninitialized scratch (no input deps, result
        # never read) so the array is at 2.4 GHz when real work starts.
        warm_w = wp.tile([P, P], bf16)
        warm_m = wp.tile([P, T], bf16)
        nc.scalar.memzero(warm_w)
        nc.scalar.memzero(warm_m)
        for i in range(18):
            warm_ps = psQ.tile([P, T], f32, tag="qs", name=f"warm_{i}")
            nc.tensor.matmul(warm_ps, warm_w, warm_m, start=True, stop=True)

        # fp16 operands first: the grouped-per-chunk matmul order runs all
        # fp16 matmuls before the fp8 corrections, so step 1 can start as
        # soon as w1h/q16s land.  All DMAs issue from the Sync engine only —
        # a DMA trigger blocked on a busy ring would stall any compute op
        # behind it in that engine's FIFO, and ACT/DVE drains are on the
        # critical path.
        # startup-only: issue from both engines in parallel (ACT has no
        # compute queued yet, so its FIFO can't block anything critical)
        for k in range(HT):
            ksl = slice(128 * k, 128 * (k + 1))
            nc.sync.dma_start(out=w1h[:, k, :], in_=W1h_d[ksl, :])
            nc.scalar.dma_start(out=q16s0[:, k, :], in_=q16s_d[0, ksl, :])
        for k in range(HT):
            ksl = slice(128 * k, 128 * (k + 1))
            nc.sync.dma_start(out=w1c[:, k, :, :], in_=W1c_d[ksl, :, :])
            nc.scalar.dma_start(out=q8s0[:, k, :, :], in_=q8_d[0, ksl, :, :])
        bo_sb = wp.tile([P, HT], f32)
        nc.sync.dma_start(out=bo_sb, in_=bo[:, :])
        mask_sb = wp.tile([P, BPC, ST], f32)
        nc.sync.dma_start(out=mask_sb, in_=maskc[:, :, :].rearrange("b p m -> p b m"))
        ones_sb = wp.tile([P, P], bf16)
        nc.sync.dma_start(out=ones_sb, in_=onesv[:, :])
        w16 = wp.tile([P, 2, HT, H], f16)

        st = {}

        def load_q16(b):
            # step-5 moving q for batch b; emitted well before C(b) so the
            # transfer isn't stuck behind a later batch's demand-paced et
            # stream
            q16 = pq16.tile([P, HT, T], f16, tag="q16")
            nc.sync.dma_start(
                out=q16,
                in_=q16_d[b, :, :].rearrange("(k p) t -> p k t", p=P))
            st[("q16", b)] = q16

        def phase_A(b):
            # --- load q side; step 1; step 2 + max tree ---
            if b == 0:
                q16s, q8s = q16s0, q8s0
            else:
                q16s = pq.tile([P, HT, T], f16, tag="q16s")
                q8s = pq.tile([P, HT, 2, T], fp8, tag="q8")
                nc.sync.dma_start(
                    out=q16s,
                    in_=q16s_d[b, :, :].rearrange("(k p) t -> p k t", p=P))
                nc.sync.dma_start(
                    out=q8s,
                    in_=q8_d[b, :, :, :].rearrange("(k p) c t -> p k c t", p=P))
            if b > 0:
                load_q16(b - 1)

            # step 1: PSUM accumulates 2^12 * qw
            qwh = pb.tile([P, HT, T], f16, tag="big2a")      # fp16(2^12 qw)
            qw8c = pb.tile([P, HT, 2, T], fp8, tag="big2b")  # [4096*qwlo, qw]
            for m in range(HT):
                qw_ps = psQ.tile([P, T], f32, tag="qs", name=f"qw_{b}_{m}")
                msl = slice(128 * m, 128 * (m + 1))
                for k in range(HT):
                    nc.tensor.matmul(qw_ps, w1h[:, k, msl], q16s[:, k, :],
                                     start=(k == 0), stop=False)
                for k in range(HT):
                    nc.tensor.matmul(qw_ps, w1c[:, k, :, msl], q8s[:, k, :, :],
                                     perf_mode=DR,
                                     start=False, stop=(k == HT - 1))
                nc.scalar.copy(qwh[:, m, :], qw_ps)
                nc.scalar.activation(qw8c[:, m, 1, :], qw_ps, AF.Copy,
                                     scale=1.0 / SC)
                nc.vector.tensor_sub(qw8c[:, m, 0, :], qw_ps, qwh[:, m, :])

            # step 2: PSUM accumulates 2^12 * score
            L = cps[b]
            score = pb.tile([P, ST, T], f32, tag="big1")
            smax = sp.tile([P, T], f32, tag="smax")
            for m2 in range((L + 1) // 2):
                ets16, ets8 = [], []
                for k in range(HT):
                    et = etp.tile([P, 2, 512], mybir.dt.uint8, tag="et")
                    nc.sync.dma_start(out=et, in_=eTp[b, k, :, m2, :, :])
                    ets16.append(et[:, 0, :].bitcast(f16))
                    ets8.append(et[:, 1, :].bitcast(fp8)
                                .rearrange("p (c x) -> p c x", c=2))
                for j in range(2):
                    m = 2 * m2 + j
                    if m >= L:
                        continue
                    jsl = slice(128 * j, 128 * (j + 1))
                    sc_ps = psQ.tile([P, T], f32, tag="qs", name=f"sc_{b}_{m}")
                    for k in range(HT):
                        nc.tensor.matmul(sc_ps, ets16[k][:, jsl], qwh[:, k, :],
                                         start=(k == 0), stop=False)
                    for k in range(HT):
                        nc.tensor.matmul(sc_ps, ets8[k][:, :, jsl],
                                         qw8c[:, k, :, :], perf_mode=DR,
                                         start=False, stop=(k == HT - 1))
                    nc.scalar.activation(score[:, m, :], sc_ps, AF.Copy,
                                         scale=1.0 / SC)
                    if m == 0:
                        nc.vector.tensor_copy(smax, score[:, m, :])
                    else:
                        nc.vector.tensor_max(smax, smax, score[:, m, :])
            st[b] = (score, smax)

        def phase_B(b):
            # --- softmax: global max, exp (-> bf16 e), denominator ---
            score, smax = st[b]
            # max over masked-but-real rows is included; the uniform upward
            # shift cancels in the softmax ratio.
            smax_all = sp1.tile([P, T], f32, tag="smax_all")
            nc.gpsimd.partition_all_reduce(smax_all, smax, channels=P,
                                           reduce_op=bass_isa.ReduceOp.max)
            L = cps[b]
            e = pb.tile([P, ST, T], bf16, tag="big2a")
            for m in range(L):
                nc.vector.tensor_sub(score[:, m, :], score[:, m, :], smax_all)
                nc.scalar.activation(e[:, m, :], score[:, m, :], AF.Exp,
                                     bias=mask_sb[:, b, m:m + 1])
            # all-ones stationary -> den replicated on all 128 partitions
            den_ps = psD.tile([P, T], f32, tag="den", name=f"den_{b}")
            for m in range(L):
                nc.tensor.matmul(den_ps, ones_sb, e[:, m, :],
                                 start=(m == 0), stop=(m == L - 1))
            rdenb = sp1.tile([P, T], f32, tag="rdenb")
            nc.vector.reciprocal(rdenb, den_ps)
            st[b] = (e, rdenb)

        def phase_C(b):
            # --- context (bf16) + output projection (fp16) ---
            L = cps[b]
            e, rdenb = st[b]
            del st[b]
            q16 = st.pop(("q16", b))
            enc_sb = enc_p.tile([P, ST, H], bf16, tag="enc")
            for k in range(L):
                nc.sync.dma_start(out=enc_sb[:, k, :],
                                  in_=encb[b, 128 * k:128 * (k + 1), :])

            cn = pb.tile([P, HT, T], f16, tag="big1")
            for m in range(HT):
                c_ps = psC.tile([P, T], f32, tag="c", name=f"c_{b}_{m}")
                for k in range(L):
                    nc.tensor.matmul(c_ps, enc_sb[:, k, 128 * m:128 * (m + 1)],
                                     e[:, k, :],
                                     start=(k == 0), stop=(k == L - 1))
                nc.vector.tensor_mul(cn[:, m, :], c_ps, rdenb)

            for m in range(HT):
                o_ps = psO.tile([P, T], f32, tag="o", name=f"o_{b}_{m}")
                msl = slice(128 * m, 128 * (m + 1))
                for k in range(HT):
                    nc.tensor.matmul(o_ps, w16[:, 0, k, msl], q16[:, k, :],
                                     start=(k == 0), stop=False)
                for k in range(HT):
                    nc.tensor.matmul(o_ps, w16[:, 1, k, msl], cn[:, k, :],
                                     start=False, stop=(k == HT - 1))
                ot = otp.tile([P, T], f32, tag="ot")
                nc.scalar.activation(ot, o_ps, AF.Tanh, bias=bo_sb[:, m:m + 1])
                nc.sync.dma_start(out=outT[b, 128 * m:128 * (m + 1), :],
                                  in_=ot)

        # software pipeline: A0 A1 B0 C0 B1 A2 C1 B2 A3 C2 B3 C3
        phase_A(0)
        nc.sync.dma_start(
            out=w16, in_=W16[:, :, :].rearrange("(k p) c o -> p c k o", p=P))
        phase_A(1)
        phase_B(0)
        phase_C(0)
        phase_B(1)
        phase_A(2)
        phase_C(1)
        phase_B(2)
        phase_A(3)
        phase_C(2)
        load_q16(3)
        phase_B(3)
        phase_C(3)

    nc.compile()
    return nc


def _f16_split(x):
    hi = x.astype(np.float16).astype(np.float32)
    return hi, x - hi


def _f8(x):
    return x.astype(ml_dtypes.float8_e4m3)


def kernel(query, encoder_outputs, src_lengths, W_in, W_out, b_out):
    query = np.asarray(query, dtype=np.float32)
    encoder_outputs = np.ascontiguousarray(np.asarray(encoder_outputs, np.float32))
    src_lengths = np.asarray(src_lengths)
    W_in = np.asarray(W_in, dtype=np.float32)
    W_out = np.asarray(W_out, dtype=np.float32)
    b_out = np.asarray(b_out, dtype=np.float32)

    # --- shared (weight) inputs ---
    W_inT = np.ascontiguousarray(W_in.T)                    # [h, o]
    _, Wlo = _f16_split(W_inT)
    W1h = W_inT.astype(np.float16)
    W1c = np.ascontiguousarray(
        np.stack([_f8(8.0 * W_inT), _f8(16384.0 * Wlo)], axis=1))  # [h, 2, o]
    W16 = np.ascontiguousarray(
        np.stack([W_out[:, :H].T, W_out[:, H:].T], axis=1)  # [h, 2, o]
    ).astype(np.float16)
    bo = np.ascontiguousarray(b_out.reshape(HT, P).T)       # [p, m]
    onesv = np.ones((P, P), dtype=ml_dtypes.bfloat16)

    # --- batch -> (core, slot) assignment: sort by length (desc) so slot j
    # holds ranks [8j, 8j+8) and the compiled per-slot chunk count is the
    # max over one octile, minimizing skipped-chunk waste ---
    lens_all = np.asarray(src_lengths, dtype=np.int64)
    order = np.argsort(-lens_all, kind="stable")            # [32] batch ids
    asg = order.reshape(BPC, NCORES)                        # [slot, core]
    cps = tuple(int(np.ceil(lens_all[asg[j]].max() / P)) for j in range(BPC))

    # --- per-core shards ---
    in_maps = []
    for c in range(NCORES):
        bidx = asg[:, c]                                    # batch ids, by slot
        q = query[bidx]                                     # [BPC, T, H]
        encs = np.ascontiguousarray(encoder_outputs[bidx])  # [BPC, S, H]
        lens = lens_all[bidx]

        qTa = np.ascontiguousarray(q.transpose(0, 2, 1))    # [BPC, H, T]
        _, qlo = _f16_split(qTa)
        q16sa = (4096.0 * qTa).astype(np.float16)           # fp16(2^12 q)
        q8a = np.ascontiguousarray(
            np.stack([_f8(512.0 * qlo), _f8(0.25 * qTa)], axis=2))  # [b,h,2,t]
        q16a = qTa.astype(np.float16)                       # [BPC, H, T]
        encTa = np.ascontiguousarray(encs.transpose(0, 2, 1))  # [BPC, H, S]
        eh, elo = _f16_split(encTa)
        # pre-tile + byte-pack fp16 hi and fp8 correction slots for one DMA
        # per (k, m2) with contiguous 1KB per-partition lines
        eT16a = eh.astype(np.float16).reshape(BPC, HT, P, ST // 2, 256)
        e8hi = _f8(encTa).reshape(BPC, HT, P, ST // 2, 256)
        e8lo = _f8(4096.0 * elo).reshape(BPC, HT, P, ST // 2, 256)
        eT8a = np.stack([e8hi, e8lo], axis=4)       # [b,k,p,m2,2,256]
        eTpa = np.empty((BPC, HT, P, ST // 2, 2, 512), dtype=np.uint8)
        eTpa[..., 0, :] = eT16a.view(np.uint8).reshape(BPC, HT, P, ST // 2, 512)
        eTpa[..., 1, :] = eT8a.view(np.uint8).reshape(BPC, HT, P, ST // 2, 512)
        encba = encs.astype(ml_dtypes.bfloat16)             # [BPC, S, H]

        maskca = np.zeros((BPC, P, ST), dtype=np.float32)
        pos = (np.arange(ST)[None, :] * P + np.arange(P)[:, None])  # [P, ST]
        for j in range(BPC):
            maskca[j][pos >= lens[j]] = MASKVAL

        in_maps.append({
            "q16s": q16sa, "q8": q8a, "q16": q16a,
            "eTp": eTpa, "encb": encba,
            "maskc": maskca, "W1h": W1h, "W1c": W1c, "W16": W16,
            "bo": bo, "onesv": onesv,
        })

    if cps not in _nc_cache_by_cps:
        _nc_cache_by_cps[cps] = _build_nc(cps)
    nc = _nc_cache_by_cps[cps]

    res = run_bass_kernel_spmd(nc, in_maps, core_ids=list(range(NCORES)),
                               trace=TRACE)
    global LAST_RESULTS
    LAST_RESULTS = res

    out = np.empty((B, T, H), dtype=np.float32)
    for c in range(NCORES):
        o = res.results[c]["outT"]                          # [BPC, H, T]
        out[asg[:, c]] = o.transpose(0, 2, 1)
    return out
